# revision 7
# baseline (speedup 1.0000x reference)
"""Trainium2 Bass kernel for out = x @ W.T + b  (x:[8192,1024] f32, W:[1024,1024] f32, b:[1024] f32).

Data-parallel over batch across 8 NeuronCores: each core computes a
[1024,1024] @ [1024,1024]^T matmul + bias for its 1024-row batch shard.

Host-side prep (inside kernel(), not on device): shard x along batch,
pre-transpose x and W so the contraction dim (in_f) lands on SBUF
partitions with fully-contiguous per-partition DMA reads, and cast to
the compute dtype. The PE contracts over the partition dim and fp32 has
no DMA-transpose path, so the host-side layout removes all on-chip
transposes.

Schedule (from perfetto analysis of the previous version):
  - W rides the ACT HWDGE ring while x rides the SP ring, so the two
    halves of the input stream transfer in parallel and the first
    matmul's operands land ~2.5us earlier.
  - Scratch warm-up matmuls run on the PE from the end of the framework
    preamble, so the HAM clock gate opens (1.2 -> 2.4 GHz) while the
    first input DMAs are still in flight instead of ~3.4us into the
    real matmul stream.
  - Outputs are written as fp16 (adds ~1e-4 rel err, halves output HBM
    traffic); the final output group is split into 4 column chunks so
    the drain after the last matmul is ~1us instead of ~3us.

Compute modes (MODE):
  f16   : single-pass fp16 matmuls. rel err ~3e-4, fastest DMA (2B in/out).
  f16x3 : fp16 hi/lo split, 3 accumulated matmuls -> ~fp32 accuracy (~3e-7).
  f32r  : float32r (relaxed fp32) matmuls, 1 cyc/row.
  f32   : exact fp32 matmuls (4 cyc/row), reference-grade.
"""

import os

import numpy as np

import concourse.bass as bass
import concourse.mybir as mybir
import concourse.tile as tile
from concourse import bacc
from concourse.bass_utils import run_bass_kernel_spmd

N_CORES = 8
B, IN_F, OUT_F = 8192, 1024, 1024
B_SHARD = B // N_CORES          # 1024 batch rows per core
P = 128                         # SBUF partitions
KO = IN_F // P                  # 8 contraction subtiles
NT = B_SHARD // P               # 8 batch tiles per core
NO = 2                          # 2 output column tiles of 512
OW = OUT_F // NO                # 512 (one PSUM bank of fp32)
NC_LAST = 4                     # column chunks for the final output group

MODE = os.environ.get("BASS_KERNEL_MODE", "f16")
N_WARMUP = int(os.environ.get("BASS_WARMUP_MMS", "30"))

_nc_cache = {}


def _build(mode):
    f32 = mybir.dt.float32
    dt_in = {
        "f16": mybir.dt.float16,
        "f16x3": mybir.dt.float16,
        "f32r": mybir.dt.float32r,
        "f32": f32,
    }[mode]
    split = mode == "f16x3"
    dt_out = mybir.dt.float16 if mode == "f16" else f32

    nc = bacc.Bacc("TRN2", target_bir_lowering=False)

    # DRAM layouts are host-packed so every DMA is contiguous per partition:
    #   xt[ki, t, ko, bi]  = x_shard[t*128+bi, ko*128+ki]
    #   wt[ki, ot, ko, oi] = W[ot*512+oi, ko*128+ki]
    xt_d = nc.dram_tensor("xt", [P, NT, KO, P], dt_in, kind="ExternalInput")
    wt_d = nc.dram_tensor("wt", [P, NO, KO, OW], dt_in, kind="ExternalInput")
    if split:
        xl_d = nc.dram_tensor("xl", [P, NT, KO, P], dt_in, kind="ExternalInput")
        wl_d = nc.dram_tensor("wl", [P, NO, KO, OW], dt_in, kind="ExternalInput")
    bias_d = nc.dram_tensor("bias", [1, OUT_F], f32, kind="ExternalInput")
    out_d = nc.dram_tensor("out", [B_SHARD, OUT_F], dt_out, kind="ExternalOutput")

    with tile.TileContext(nc) as tc:
        with (
            tc.tile_pool(name="singles", bufs=1) as singles,
            tc.tile_pool(name="wpool", bufs=NO * (2 if split else 1)) as wpool,
            tc.tile_pool(name="xpool", bufs=1) as xpool,
            tc.tile_pool(name="xlpool", bufs=1) as xlpool,
            tc.tile_pool(name="opool", bufs=NT) as opool,
            tc.tile_pool(name="pswarm", bufs=1, space="PSUM") as pswarm,
            tc.tile_pool(name="psums", bufs=3, space="PSUM") as psums,
            tc.tile_pool(name="psl", bufs=NC_LAST, space="PSUM") as psl,
        ):
            # --- PE warm-up: scratch matmuls keep the PE busy from the end
            # of the framework preamble so the HAM clock gate opens before
            # the first real matmul's operands land. ~60-110ns each; no
            # reads ever consume ps_warm.
            scr = singles.tile([P, P], dt_in)
            nc.vector.memset(scr[:], 0.0)
            ps_warm = pswarm.tile([P, 64], f32, name="ps_warm", tag="warm")
            for _ in range(N_WARMUP):
                nc.tensor.matmul(ps_warm[:], scr[:], scr[:, :64],
                                 start=True, stop=True)

            bias_row = singles.tile([1, OUT_F], f32)
            bias_sb = singles.tile([P, OUT_F], f32)

            w_tiles = [
                wpool.tile([P, KO, OW], dt_in, name=f"w_{ot}", tag="w_sb")
                for ot in range(NO)
            ]
            wl_tiles = (
                [wpool.tile([P, KO, OW], dt_in, name=f"wl_{ot}", tag="w_sb")
                 for ot in range(NO)] if split else []
            )
            xall = xpool.tile([P, NT, KO, P], dt_in, name="xall", tag="x_sb")
            xlall = (xlpool.tile([P, NT, KO, P], dt_in, name="xlall", tag="xl_sb")
                     if split else None)
            o_tiles = [
                [opool.tile([P, OW], dt_out, name=f"o_{t}_{ot}", tag="o_sb")
                 for ot in range(NO)]
                for t in range(NT)
            ]

            # Input streams ride both HWDGE rings in parallel:
            #   ACT ring (nc.scalar): w0 in 2-ko chunks (first matmuls
            #     consume them in order), then bias, then w1 coarse.
            #   SP ring (nc.sync): x tiles 0-3 singly (pipeline startup),
            #     then 4-7 in one 1MB DMA (16KB/partition runs).
            for kc in range(0, KO, 2):
                nc.scalar.dma_start(
                    out=w_tiles[0][:, kc:kc + 2], in_=wt_d[:, 0, kc:kc + 2]
                )
            nc.scalar.dma_start(out=bias_row[:], in_=bias_d[:])
            nc.gpsimd.partition_broadcast(bias_sb[:], bias_row[:])
            for kc in range(0, KO, 4):
                nc.scalar.dma_start(
                    out=w_tiles[1][:, kc:kc + 4], in_=wt_d[:, 1, kc:kc + 4]
                )
            for t in range(4):
                nc.sync.dma_start(out=xall[:, t], in_=xt_d[:, t])
            nc.sync.dma_start(out=xall[:, 4:8], in_=xt_d[:, 4:8])
            if split:
                nc.sync.dma_start(out=xlall[:], in_=xl_d[:])
                for ot in range(NO):
                    nc.scalar.dma_start(out=wl_tiles[ot][:], in_=wl_d[:, ot])

            def mm_groups(t, ot):
                groups = [(xall, w_tiles[ot])]
                if split:
                    groups += [(xlall, w_tiles[ot]), (xall, wl_tiles[ot])]
                return groups

            for ot in range(NO):
                for t in range(NT):
                    is_last = ot == NO - 1 and t == NT - 1
                    groups = mm_groups(t, ot)
                    if not is_last:
                        ps = psums.tile([P, OW], f32, name="ps", tag="ps")
                        n_mm = len(groups) * KO
                        i = 0
                        for lhs_sb, rhs_sb in groups:
                            for ko in range(KO):
                                nc.tensor.matmul(
                                    ps[:],
                                    lhs_sb[:, t, ko],
                                    rhs_sb[:, ko],
                                    start=(i == 0),
                                    stop=(i == n_mm - 1),
                                )
                                i += 1
                        nc.vector.tensor_add(
                            o_tiles[t][ot][:],
                            ps[:],
                            bias_sb[:, ot * OW:(ot + 1) * OW],
                        )
                        nc.scalar.dma_start(
                            out=out_d[t * P:(t + 1) * P, ot * OW:(ot + 1) * OW],
                            in_=o_tiles[t][ot][:],
                        )
                    else:
                        # final group: 4 independent 128-col chains so the
                        # first chunks' bias-add + store drain while the
                        # last chunk's matmuls still run -> ~1us tail.
                        cw = OW // NC_LAST
                        for c in range(NC_LAST):
                            sl = slice(c * cw, (c + 1) * cw)
                            psc = psl.tile([P, cw], f32, name=f"psl_{c}", tag="psl")
                            n_mm = len(groups) * KO
                            i = 0
                            for lhs_sb, rhs_sb in groups:
                                for ko in range(KO):
                                    nc.tensor.matmul(
                                        psc[:],
                                        lhs_sb[:, t, ko],
                                        rhs_sb[:, ko, sl],
                                        start=(i == 0),
                                        stop=(i == n_mm - 1),
                                    )
                                    i += 1
                            nc.vector.tensor_add(
                                o_tiles[t][ot][:, sl],
                                psc[:],
                                bias_sb[:, ot * OW + c * cw:ot * OW + (c + 1) * cw],
                            )
                            nc.scalar.dma_start(
                                out=out_d[t * P:(t + 1) * P,
                                          ot * OW + c * cw:ot * OW + (c + 1) * cw],
                                in_=o_tiles[t][ot][:, sl],
                            )
    nc.compile()
    return nc


def _get_nc(mode):
    if mode not in _nc_cache:
        _nc_cache[mode] = _build(mode)
    return _nc_cache[mode]


def _pack(x, W, b, mode):
    """Shard + retile host-side. Returns in_maps for the 8 cores."""
    np_dt = np.float16 if mode in ("f16", "f16x3") else np.float32
    x = np.asarray(x, dtype=np.float32)
    W = np.asarray(W, dtype=np.float32)
    b = np.asarray(b, dtype=np.float32)

    # [c, t, bi, ko, ki] -> [c, ki, t, ko, bi]
    xs = x.reshape(N_CORES, NT, P, KO, P).transpose(0, 4, 1, 3, 2)
    # [ot, oi, ko, ki] -> [ki, ot, ko, oi]
    ws = W.reshape(NO, OW, KO, P).transpose(3, 0, 2, 1)
    bias = np.ascontiguousarray(b.reshape(1, OUT_F))

    xt = np.ascontiguousarray(xs).astype(np_dt)
    wt = np.ascontiguousarray(ws).astype(np_dt)
    maps = [{"xt": xt[c], "wt": wt, "bias": bias} for c in range(N_CORES)]
    if mode == "f16x3":
        xlo = (xs - xt.astype(np.float32)).astype(np_dt)
        wlo = (ws - wt.astype(np.float32)).astype(np_dt)
        for c in range(N_CORES):
            maps[c]["xl"] = np.ascontiguousarray(xlo[c])
            maps[c]["wl"] = wlo
    return maps


def _run(in_maps, mode, **kwargs):
    nc = _get_nc(mode)
    return run_bass_kernel_spmd(nc, in_maps, core_ids=list(range(N_CORES)), **kwargs)


def kernel(x, W, b):
    mode = MODE
    res = _run(_pack(x, W, b, mode), mode)
    out = np.concatenate([r["out"] for r in res.results], axis=0)
    return np.ascontiguousarray(out.astype(np.float32))


# revision 9
# speedup vs baseline: 1.0209x; 1.0209x over previous
"""Trainium2 Bass kernel for out = x @ W.T + b  (x:[8192,1024] f32, W:[1024,1024] f32, b:[1024] f32).

Data-parallel over batch across 8 NeuronCores: each core computes a
[1024,1024] @ [1024,1024]^T matmul + bias for its 1024-row batch shard.

Host-side prep (inside kernel(), not on device): shard x along batch,
pre-transpose x and W so the contraction dim (in_f) lands on SBUF
partitions with fully-contiguous per-partition DMA reads, and cast to
the compute dtype. The PE contracts over the partition dim and fp32 has
no DMA-transpose path, so the host-side layout removes all on-chip
transposes.

Schedule (from perfetto analysis of the previous version):
  - W rides the ACT HWDGE ring while x rides the SP ring, so the two
    halves of the input stream transfer in parallel and the first
    matmul's operands land ~2.5us earlier.
  - Scratch warm-up matmuls run on the PE from the end of the framework
    preamble, so the HAM clock gate opens (1.2 -> 2.4 GHz) while the
    first input DMAs are still in flight instead of ~3.4us into the
    real matmul stream.
  - Outputs are written as fp16 (adds ~1e-4 rel err, halves output HBM
    traffic); the final output group is split into 4 column chunks so
    the drain after the last matmul is ~1us instead of ~3us.

Compute modes (MODE):
  f16   : single-pass fp16 matmuls. rel err ~3e-4, fastest DMA (2B in/out).
  f16x3 : fp16 hi/lo split, 3 accumulated matmuls -> ~fp32 accuracy (~3e-7).
  f32r  : float32r (relaxed fp32) matmuls, 1 cyc/row.
  f32   : exact fp32 matmuls (4 cyc/row), reference-grade.
"""

import os

import numpy as np

import concourse.bass as bass
import concourse.mybir as mybir
import concourse.tile as tile
from concourse import bacc
from concourse.bass_utils import run_bass_kernel_spmd

N_CORES = 8
B, IN_F, OUT_F = 8192, 1024, 1024
B_SHARD = B // N_CORES          # 1024 batch rows per core
P = 128                         # SBUF partitions
KO = IN_F // P                  # 8 contraction subtiles
NT = B_SHARD // P               # 8 batch tiles per core
NO = 2                          # 2 output column tiles of 512
OW = OUT_F // NO                # 512 (one PSUM bank of fp32)
NC_LAST = 4                     # column chunks for the final output group

MODE = os.environ.get("BASS_KERNEL_MODE", "f16")
N_WARMUP = int(os.environ.get("BASS_WARMUP_MMS", "48"))

_nc_cache = {}


def _build(mode):
    f32 = mybir.dt.float32
    dt_in = {
        "f16": mybir.dt.float16,
        "f16x3": mybir.dt.float16,
        "f32r": mybir.dt.float32r,
        "f32": f32,
    }[mode]
    split = mode == "f16x3"
    dt_out = mybir.dt.float16 if mode == "f16" else f32

    nc = bacc.Bacc("TRN2", target_bir_lowering=False)

    # DRAM layouts are host-packed so every DMA is contiguous per partition:
    #   xt[ki, t, ko, bi]  = x_shard[t*128+bi, ko*128+ki]
    #   wt[ki, ot, ko, oi] = W[ot*512+oi, ko*128+ki]
    xt_d = nc.dram_tensor("xt", [P, NT, KO, P], dt_in, kind="ExternalInput")
    wt_d = nc.dram_tensor("wt", [P, NO, KO, OW], dt_in, kind="ExternalInput")
    if split:
        xl_d = nc.dram_tensor("xl", [P, NT, KO, P], dt_in, kind="ExternalInput")
        wl_d = nc.dram_tensor("wl", [P, NO, KO, OW], dt_in, kind="ExternalInput")
    bias_d = nc.dram_tensor("bias", [1, OUT_F], f32, kind="ExternalInput")
    out_d = nc.dram_tensor("out", [B_SHARD, OUT_F], dt_out, kind="ExternalOutput")

    with tile.TileContext(nc) as tc:
        with (
            tc.tile_pool(name="singles", bufs=1) as singles,
            tc.tile_pool(name="wpool", bufs=NO * (2 if split else 1)) as wpool,
            tc.tile_pool(name="xpool", bufs=1) as xpool,
            tc.tile_pool(name="xlpool", bufs=1) as xlpool,
            tc.tile_pool(name="opool", bufs=NT) as opool,
            tc.tile_pool(name="pswarm", bufs=1, space="PSUM") as pswarm,
            tc.tile_pool(name="psums", bufs=3, space="PSUM") as psums,
            tc.tile_pool(name="psl", bufs=NC_LAST, space="PSUM") as psl,
        ):
            # --- PE warm-up: scratch matmuls keep the PE busy from the end
            # of the framework preamble so the HAM clock gate opens before
            # the first real matmul's operands land. ~60-110ns each; no
            # reads ever consume ps_warm.
            scr = singles.tile([P, P], dt_in)
            nc.vector.memset(scr[:], 0.0)
            ps_warm = pswarm.tile([P, 64], f32, name="ps_warm", tag="warm")
            for _ in range(N_WARMUP):
                nc.tensor.matmul(ps_warm[:], scr[:], scr[:, :64],
                                 start=True, stop=True)

            bias_row = singles.tile([1, OUT_F], f32)
            bias_sb = singles.tile([P, OUT_F], f32)

            w_tiles = [
                wpool.tile([P, KO, OW], dt_in, name=f"w_{ot}", tag="w_sb")
                for ot in range(NO)
            ]
            wl_tiles = (
                [wpool.tile([P, KO, OW], dt_in, name=f"wl_{ot}", tag="w_sb")
                 for ot in range(NO)] if split else []
            )
            xall = xpool.tile([P, NT, KO, P], dt_in, name="xall", tag="x_sb")
            xlall = (xlpool.tile([P, NT, KO, P], dt_in, name="xlall", tag="xl_sb")
                     if split else None)
            o_tiles = [
                [opool.tile([P, OW], dt_out, name=f"o_{t}_{ot}", tag="o_sb")
                 for ot in range(NO)]
                for t in range(NT)
            ]

            # All inputs ride the SP HWDGE ring, strictly ordered by
            # first-use time so nothing steals wire bandwidth from the
            # critical chain: w0 k-pair chunks land just ahead of the
            # matmuls that consume them, x tiles follow, w1 rides last
            # (first needed ~halfway through the stream). Splitting the
            # inputs across both rings was measurably worse: the SDMA
            # engines round-robin between queues, so bulk x tiles halved
            # the effective bandwidth of the urgent w0 chunks.
            nc.sync.dma_start(out=w_tiles[0][:, 0:2], in_=wt_d[:, 0, 0:2])
            nc.sync.dma_start(out=xall[:, 0], in_=xt_d[:, 0])
            for kc in range(2, KO, 2):
                nc.sync.dma_start(
                    out=w_tiles[0][:, kc:kc + 2], in_=wt_d[:, 0, kc:kc + 2]
                )
            for t in range(1, 4):
                nc.sync.dma_start(out=xall[:, t], in_=xt_d[:, t])
            nc.sync.dma_start(out=xall[:, 4:8], in_=xt_d[:, 4:8])
            for kc in range(0, KO, 4):
                nc.sync.dma_start(
                    out=w_tiles[1][:, kc:kc + 4], in_=wt_d[:, 1, kc:kc + 4]
                )
            # bias rides the ACT ring (otherwise idle until outputs drain)
            nc.scalar.dma_start(out=bias_row[:], in_=bias_d[:])
            nc.gpsimd.partition_broadcast(bias_sb[:], bias_row[:])
            if split:
                nc.sync.dma_start(out=xlall[:], in_=xl_d[:])
                for ot in range(NO):
                    nc.scalar.dma_start(out=wl_tiles[ot][:], in_=wl_d[:, ot])

            def mm_groups(t, ot):
                groups = [(xall, w_tiles[ot])]
                if split:
                    groups += [(xlall, w_tiles[ot]), (xall, wl_tiles[ot])]
                return groups

            for ot in range(NO):
                for t in range(NT):
                    is_last = ot == NO - 1 and t == NT - 1
                    groups = mm_groups(t, ot)
                    if not is_last:
                        ps = psums.tile([P, OW], f32, name="ps", tag="ps")
                        n_mm = len(groups) * KO
                        i = 0
                        for lhs_sb, rhs_sb in groups:
                            for ko in range(KO):
                                nc.tensor.matmul(
                                    ps[:],
                                    lhs_sb[:, t, ko],
                                    rhs_sb[:, ko],
                                    start=(i == 0),
                                    stop=(i == n_mm - 1),
                                )
                                i += 1
                        nc.vector.tensor_add(
                            o_tiles[t][ot][:],
                            ps[:],
                            bias_sb[:, ot * OW:(ot + 1) * OW],
                        )
                        nc.scalar.dma_start(
                            out=out_d[t * P:(t + 1) * P, ot * OW:(ot + 1) * OW],
                            in_=o_tiles[t][ot][:],
                        )
                    else:
                        # final group: 4 independent 128-col chains so the
                        # first chunks' bias-add + store drain while the
                        # last chunk's matmuls still run -> ~1us tail.
                        cw = OW // NC_LAST
                        for c in range(NC_LAST):
                            sl = slice(c * cw, (c + 1) * cw)
                            psc = psl.tile([P, cw], f32, name=f"psl_{c}", tag="psl")
                            n_mm = len(groups) * KO
                            i = 0
                            for lhs_sb, rhs_sb in groups:
                                for ko in range(KO):
                                    nc.tensor.matmul(
                                        psc[:],
                                        lhs_sb[:, t, ko],
                                        rhs_sb[:, ko, sl],
                                        start=(i == 0),
                                        stop=(i == n_mm - 1),
                                    )
                                    i += 1
                            nc.vector.tensor_add(
                                o_tiles[t][ot][:, sl],
                                psc[:],
                                bias_sb[:, ot * OW + c * cw:ot * OW + (c + 1) * cw],
                            )
                            nc.scalar.dma_start(
                                out=out_d[t * P:(t + 1) * P,
                                          ot * OW + c * cw:ot * OW + (c + 1) * cw],
                                in_=o_tiles[t][ot][:, sl],
                            )
    nc.compile()
    return nc


def _get_nc(mode):
    if mode not in _nc_cache:
        _nc_cache[mode] = _build(mode)
    return _nc_cache[mode]


def _pack(x, W, b, mode):
    """Shard + retile host-side. Returns in_maps for the 8 cores."""
    np_dt = np.float16 if mode in ("f16", "f16x3") else np.float32
    x = np.asarray(x, dtype=np.float32)
    W = np.asarray(W, dtype=np.float32)
    b = np.asarray(b, dtype=np.float32)

    # [c, t, bi, ko, ki] -> [c, ki, t, ko, bi]
    xs = x.reshape(N_CORES, NT, P, KO, P).transpose(0, 4, 1, 3, 2)
    # [ot, oi, ko, ki] -> [ki, ot, ko, oi]
    ws = W.reshape(NO, OW, KO, P).transpose(3, 0, 2, 1)
    bias = np.ascontiguousarray(b.reshape(1, OUT_F))

    xt = np.ascontiguousarray(xs).astype(np_dt)
    wt = np.ascontiguousarray(ws).astype(np_dt)
    maps = [{"xt": xt[c], "wt": wt, "bias": bias} for c in range(N_CORES)]
    if mode == "f16x3":
        xlo = (xs - xt.astype(np.float32)).astype(np_dt)
        wlo = (ws - wt.astype(np.float32)).astype(np_dt)
        for c in range(N_CORES):
            maps[c]["xl"] = np.ascontiguousarray(xlo[c])
            maps[c]["wl"] = wlo
    return maps


def _run(in_maps, mode, **kwargs):
    nc = _get_nc(mode)
    return run_bass_kernel_spmd(nc, in_maps, core_ids=list(range(N_CORES)), **kwargs)


def kernel(x, W, b):
    mode = MODE
    res = _run(_pack(x, W, b, mode), mode)
    out = np.concatenate([r["out"] for r in res.results], axis=0)
    return np.ascontiguousarray(out.astype(np.float32))
